# revision 1
# baseline (speedup 1.0000x reference)
"""Trainium2 Bass kernel for BasicMoE.

Reference computation (N=8192 tokens, D=1024 in, O=1024 out, E=8 experts):
    gates = softmax(x @ Wg + bg)                        # [N, E]
    out   = sum_e gates[:, e] * (x @ We[e] + be[e])     # [N, O]

Strategy: data-parallel over tokens. Each of the 8 NeuronCores gets a
1024-token shard of x plus the full (replicated) expert/gating weights and
computes its shard of the output. No collectives.

Per-core kernel (all matmuls bf16 inputs, f32 PSUM accumulate):
  - x shard is pre-transposed on host to xt[p, k*1024 + n] = x[n, k*128+p]
    so 128x128 lhsT tiles slice straight out of SBUF.
  - gating: z[t] = x_t @ Wg + bg via PE, softmax on ACT/DVE
    (exp with accum_out gives the row sums for free).
  - main: for e, t: psum[t,j] = sum_k xt_tile.T @ We_tile; then one fused
    DVE op acc = psum * g[:, e] + acc   (scalar_tensor_tensor).
  - bias: gT = transpose(g) on PE, psum_b = gT.T @ be (= g @ be), added
    into acc at the end.
"""

import numpy as np
import ml_dtypes

N_TOKENS = 8192
D = 1024   # in dim
O = 1024   # out dim
E = 8      # experts
NCORES = 8
NLOC = N_TOKENS // NCORES   # 1024 tokens per core
KT = D // 128               # 8 k-chunks
TT = NLOC // 128            # 8 token chunks
JT = O // 512               # 2 out chunks

BF16 = ml_dtypes.bfloat16

_CACHE = {}


def _build():
    """Build + compile the per-core Bass graph (same graph on all 8 cores)."""
    import concourse.bass as bass
    import concourse.mybir as mybir
    import concourse.tile as tile
    from concourse import bacc
    from concourse.masks import make_identity

    dt = mybir.dt
    f32 = dt.float32
    bf16 = dt.bfloat16
    Alu = mybir.AluOpType

    nc = bacc.Bacc(
        "TRN2",
        target_bir_lowering=False,
        debug=False,
        enable_asserts=False,
        num_devices=NCORES,
    )

    xt_d = nc.dram_tensor("xt", [128, KT * NLOC], bf16, kind="ExternalInput").ap()
    we_d = nc.dram_tensor("Wep", [E, 128, KT * O], bf16, kind="ExternalInput").ap()
    be_d = nc.dram_tensor("bep", [E, O], bf16, kind="ExternalInput").ap()
    wg_d = nc.dram_tensor("Wgp", [128, KT * E], bf16, kind="ExternalInput").ap()
    bg_d = nc.dram_tensor("bgp", [1, E], bf16, kind="ExternalInput").ap()
    out_d = nc.dram_tensor("out", [NLOC, O], f32, kind="ExternalOutput").ap()

    with tile.TileContext(nc) as tc:
        with (
            tc.tile_pool(name="const", bufs=1) as cpool,
            tc.tile_pool(name="xp", bufs=1) as xpool,
            tc.tile_pool(name="wp", bufs=3) as wpool,
            tc.tile_pool(name="ap", bufs=1) as apool,
            tc.tile_pool(name="gp", bufs=1) as gpool,
        ):
            ident = cpool.tile([128, 128], bf16)
            make_identity(nc, ident[:])
            ones = cpool.tile([1, 128], bf16)
            nc.gpsimd.memset(ones[:], 1.0)
            # Small gating/bias constants go on the SWDGE (gpsimd) queue so
            # they don't serialize behind xt on the sync HWDGE ring.
            wg_sb = cpool.tile([128, KT * E], bf16)
            nc.gpsimd.dma_start(wg_sb[:], wg_d)
            bg_sb = cpool.tile([1, E], bf16)
            nc.gpsimd.dma_start(bg_sb[:], bg_d)
            be_sb = cpool.tile([E, O], bf16)
            nc.gpsimd.dma_start(be_sb[:], be_d)

            # xt first, split across both HWDGE rings so the two halves
            # stream concurrently; everything downstream needs it.
            xt = xpool.tile([128, KT * NLOC], bf16)
            half = KT * NLOC // 2
            nc.sync.dma_start(xt[:, :half], xt_d[:, :half])
            nc.scalar.dma_start(xt[:, half:], xt_d[:, half:])

            acc = apool.tile([128, TT * O], f32)

            g_f32 = gpool.tile([128, TT * E], f32)
            g_bf = gpool.tile([128, TT * E], bf16)
            gT = gpool.tile([E, NLOC], bf16)
            negm = gpool.tile([128, TT], f32)
            ssum = gpool.tile([128, TT], f32)
            rec = gpool.tile([128, TT], f32)

            def xt_tile(k, t):
                c = k * NLOC + t * 128
                return xt[:, c : c + 128]

            # Expert weights on the same sync ring as xt: HWDGE drains FIFO,
            # so xt gets full HBM bandwidth first, then We[0], We[1], ... in
            # exactly the order compute consumes them. Each expert arrives as
            # two j-half DMAs so e=0 can start on the first half.
            we_tiles = []
            for e in range(E):
                we_sb = wpool.tile([128, KT * O], bf16, tag="we", name=f"we{e}")
                src = we_d[e].rearrange("p (k j c) -> j p k c", k=KT, j=JT, c=512)
                dst = we_sb.rearrange("p (k j c) -> j p k c", k=KT, j=JT, c=512)
                for jh in range(JT):
                    nc.sync.dma_start(dst[jh], src[jh])
                we_tiles.append(we_sb)

            # ---- Phase A: gating logits + softmax --------------------------
            with tc.tile_pool(name="psA", bufs=2, space="PSUM") as psA:
                for t in range(TT):
                    zg = psA.tile([128, E], f32, tag="zg")
                    for k in range(KT):
                        nc.tensor.matmul(
                            zg[:],
                            xt_tile(k, t),
                            wg_sb[:, k * E : (k + 1) * E],
                            start=(k == 0),
                            stop=False,
                        )
                    # + bg (rank-1: ones[1,128].T @ bg[1,E])
                    nc.tensor.matmul(zg[:], ones[:], bg_sb[:], start=False, stop=True)

                    nm = negm[:, t : t + 1]
                    nc.vector.tensor_reduce(
                        nm, zg[:], axis=mybir.AxisListType.X, op=Alu.max, negate=True
                    )
                    gs = g_f32[:, t * E : (t + 1) * E]
                    nc.scalar.activation(
                        gs,
                        zg[:],
                        mybir.ActivationFunctionType.Exp,
                        bias=nm,
                        scale=1.0,
                        accum_out=ssum[:, t : t + 1],
                    )
                    nc.vector.reciprocal(rec[:, t : t + 1], ssum[:, t : t + 1])
                    nc.vector.tensor_scalar_mul(gs, gs, rec[:, t : t + 1])
                    nc.vector.tensor_copy(g_bf[:, t * E : (t + 1) * E], gs)

            # ---- Phase A2: transpose gates for the bias matmul -------------
            with tc.tile_pool(name="psC", bufs=1, space="PSUM") as psC:
                for t in range(TT):
                    trp = psC.tile([E, 128], bf16, tag="tr")
                    nc.tensor.transpose(
                        trp[:], g_bf[:, t * E : (t + 1) * E], ident[:]
                    )
                    nc.vector.tensor_copy(gT[:, t * 128 : (t + 1) * 128], trp[:])

            # ---- Phase B: expert GEMMs + gated accumulate ------------------
            # e == 0 writes acc (no init needed); e >= 1 run the fused DVE
            # accumulate acc = psum_e * g_e + acc; the bias term g @ be is
            # folded in during the last expert's pass.
            with (
                tc.tile_pool(name="psD", bufs=2, space="PSUM") as psD,
                tc.tile_pool(name="psB", bufs=6, space="PSUM") as psB,
            ):
                # e = 0: j-outer so the j=0 half of We[0] is consumed as soon
                # as its DMA lands, ~3us before the j=1 half.
                for j in range(JT):
                    we_sb = we_tiles[0]
                    for t in range(TT):
                        ps0 = psB.tile([128, 512], f32, tag="mm", name="mm0")
                        for k in range(KT):
                            nc.tensor.matmul(
                                ps0[:],
                                xt_tile(k, t),
                                we_sb[:, k * O + j * 512 : k * O + (j + 1) * 512],
                                start=(k == 0),
                                stop=(k == KT - 1),
                            )
                        nc.vector.tensor_scalar_mul(
                            acc[:, t * O + j * 512 : t * O + (j + 1) * 512],
                            ps0[:],
                            g_f32[:, t * E : t * E + 1],
                        )

                for e in range(1, E):
                    we_sb = we_tiles[e]
                    last = e == E - 1
                    for t in range(TT):
                        bps = []
                        if last:
                            # Bias term g @ be, folded into the final pass so
                            # its PSUM tiles are short-lived.
                            for j in range(JT):
                                bp = psD.tile([128, 512], f32, tag="bp", name=f"bp{j}")
                                nc.tensor.matmul(
                                    bp[:],
                                    gT[:, t * 128 : (t + 1) * 128],
                                    be_sb[:, j * 512 : (j + 1) * 512],
                                    start=True,
                                    stop=True,
                                )
                                bps.append(bp)
                        ps = [
                            psB.tile([128, 512], f32, tag="mm", name=f"mm{j}")
                            for j in range(JT)
                        ]
                        for k in range(KT):
                            lhs = xt_tile(k, t)
                            for j in range(JT):
                                nc.tensor.matmul(
                                    ps[j][:],
                                    lhs,
                                    we_sb[:, k * O + j * 512 : k * O + (j + 1) * 512],
                                    start=(k == 0),
                                    stop=(k == KT - 1),
                                )
                        gcol = g_f32[:, t * E + e : t * E + e + 1]
                        for j in range(JT):
                            a_sl = acc[:, t * O + j * 512 : t * O + (j + 1) * 512]
                            if last:
                                # Fold the bias in BEFORE the final expert's
                                # accumulate so the post-last-matmul critical
                                # path is one DVE op + the store.
                                nc.vector.scalar_tensor_tensor(
                                    a_sl, bps[j][:], 1.0, a_sl,
                                    op0=Alu.mult, op1=Alu.add,
                                )
                            nc.vector.scalar_tensor_tensor(
                                a_sl, ps[j][:], gcol, a_sl,
                                op0=Alu.mult, op1=Alu.add,
                            )
                            if last:
                                nc.sync.dma_start(
                                    out_d[
                                        t * 128 : (t + 1) * 128,
                                        j * 512 : (j + 1) * 512,
                                    ],
                                    a_sl,
                                )

    nc.compile()
    return nc


def _get_nc():
    if "nc" not in _CACHE:
        _CACHE["nc"] = _build()
    return _CACHE["nc"]


def _pack_inputs(x, We, be, Wg, bg):
    """Host-side packing: shard + pre-transpose + bf16 cast."""
    x = np.asarray(x, dtype=np.float32)
    We = np.asarray(We, dtype=np.float32)
    be = np.asarray(be, dtype=np.float32)
    Wg = np.asarray(Wg, dtype=np.float32)
    bg = np.asarray(bg, dtype=np.float32)

    # [p, k*O + o] = We[e][k*128+p, o]
    we_p = np.ascontiguousarray(
        We.reshape(E, KT, 128, O).transpose(0, 2, 1, 3).reshape(E, 128, KT * O)
    ).astype(BF16)
    be_p = be.astype(BF16)
    wg_p = np.ascontiguousarray(
        Wg.reshape(KT, 128, E).transpose(1, 0, 2).reshape(128, KT * E)
    ).astype(BF16)
    bg_p = bg.reshape(1, E).astype(BF16)

    in_maps = []
    for i in range(NCORES):
        xs = x[i * NLOC : (i + 1) * NLOC]          # [NLOC, D]
        # xt[p, k*NLOC + n] = xs[n, k*128+p]
        xt = np.ascontiguousarray(
            xs.T.reshape(KT, 128, NLOC).transpose(1, 0, 2).reshape(128, KT * NLOC)
        ).astype(BF16)
        in_maps.append(
            {"xt": xt, "Wep": we_p, "bep": be_p, "Wgp": wg_p, "bgp": bg_p}
        )
    return in_maps


def _run(inputs, trace=False):
    """Returns (y_full, BassKernelResults)."""
    from concourse.bass_utils import run_bass_kernel_spmd

    nc = _get_nc()
    in_maps = _pack_inputs(**inputs)
    res = run_bass_kernel_spmd(
        nc, in_maps, core_ids=list(range(NCORES)), trace=trace
    )
    y = np.concatenate(
        [res.results[i]["out"] for i in range(NCORES)], axis=0
    ).astype(np.float32)
    return y, res


def kernel(**inputs):
    y, _ = _run(inputs, trace=False)
    return y



# revision 3
# speedup vs baseline: 1.0039x; 1.0039x over previous
"""Trainium2 Bass kernel for BasicMoE.

Reference computation (N=8192 tokens, D=1024 in, O=1024 out, E=8 experts):
    gates = softmax(x @ Wg + bg)                        # [N, E]
    out   = sum_e gates[:, e] * (x @ We[e] + be[e])     # [N, O]

Strategy: data-parallel over tokens. Each of the 8 NeuronCores gets a
1024-token shard of x plus the full (replicated) expert/gating weights and
computes its shard of the output. No collectives.

All host-side packing produces layouts whose DMAs are 128 rows x >=2KB
contiguous descriptors (the strided layouts of v1 cost ~20us of descriptor
generation on the HWDGE ring and delayed phase B to ~28us).

Per-core kernel (all matmuls bf16 inputs, f32 PSUM accumulate):
  - x shard pre-transposed t-major: 8 chunks of [128, KT*128]; gating for
    chunk t starts as soon as its 256KB DMA lands.
  - scalar ring: Wg, bg, then the 8 xt chunks.  sync ring: the 16 We
    (e, j-half) 1MB blocks in consumption order.  gpsimd: be.
  - gating: z[t] = x_t @ Wg + bg on PE, softmax on ACT/DVE.
  - main: for e, t: psum[t,j] = sum_k xt_tile.T @ We_tile; then one fused
    DVE op acc = psum * g[:, e] + acc   (scalar_tensor_tensor).
  - bias: gT = transpose(g) on PE, psum_b = gT.T @ be (= g @ be), added
    into acc during the last expert's pass.
"""

import numpy as np
import ml_dtypes

N_TOKENS = 8192
D = 1024   # in dim
O = 1024   # out dim
E = 8      # experts
NCORES = 8
NLOC = N_TOKENS // NCORES   # 1024 tokens per core
KT = D // 128               # 8 k-chunks
TT = NLOC // 128            # 8 token chunks
JT = O // 512               # 2 out chunks

BF16 = ml_dtypes.bfloat16

_CACHE = {}


def _build():
    """Build + compile the per-core Bass graph (same graph on all 8 cores)."""
    import concourse.bass as bass
    import concourse.mybir as mybir
    import concourse.tile as tile
    from concourse import bacc
    from concourse.masks import make_identity

    dt = mybir.dt
    f32 = dt.float32
    bf16 = dt.bfloat16
    Alu = mybir.AluOpType

    nc = bacc.Bacc(
        "TRN2",
        target_bir_lowering=False,
        debug=False,
        enable_asserts=False,
        num_devices=NCORES,
    )

    # xt: t-major — chunk t is [128, KT*128], contiguous 2KB rows.
    xt_d = nc.dram_tensor("xt", [128, TT * KT * 128], bf16, kind="ExternalInput").ap()
    # We: [e, j, p, k*512+c] — each (e, j) block is [128, KT*512], 8KB rows.
    we_d = nc.dram_tensor("Wep", [E, JT, 128, KT * 512], bf16, kind="ExternalInput").ap()
    be_d = nc.dram_tensor("bep", [E, O], bf16, kind="ExternalInput").ap()
    wg_d = nc.dram_tensor("Wgp", [128, KT * E], bf16, kind="ExternalInput").ap()
    bg_d = nc.dram_tensor("bgp", [1, E], bf16, kind="ExternalInput").ap()
    out_d = nc.dram_tensor("out", [NLOC, O], f32, kind="ExternalOutput").ap()

    with tile.TileContext(nc) as tc:
        with (
            tc.tile_pool(name="const", bufs=1) as cpool,
            tc.tile_pool(name="xp", bufs=TT) as xpool,
            tc.tile_pool(name="wp", bufs=6) as wpool,
            tc.tile_pool(name="ap", bufs=1) as apool,
            tc.tile_pool(name="gp", bufs=1) as gpool,
        ):
            ident = cpool.tile([128, 128], bf16)
            make_identity(nc, ident[:])
            ones = cpool.tile([1, 128], bf16)
            nc.gpsimd.memset(ones[:], 1.0)

            # Gating weights first on the scalar ring (tiny; gating needs
            # them before the first xt chunk lands).
            wg_sb = cpool.tile([128, KT * E], bf16)
            nc.scalar.dma_start(wg_sb[:], wg_d)
            bg_sb = cpool.tile([1, E], bf16)
            nc.scalar.dma_start(bg_sb[:], bg_d)

            # xt chunks stream on the scalar ring right behind Wg/bg.
            xts = []
            for t in range(TT):
                xc = xpool.tile([128, KT * 128], bf16, tag="xt", name=f"xt{t}")
                nc.scalar.dma_start(
                    xc[:], xt_d[:, t * KT * 128 : (t + 1) * KT * 128]
                )
                xts.append(xc)

            # be on the SWDGE queue (only needed for the bias pass at e=7).
            be_sb = cpool.tile([E, O], bf16)
            nc.gpsimd.dma_start(be_sb[:], be_d)

            # Expert weights on the sync ring in exact consumption order;
            # each (e, j) block is one 1MB DMA with 8KB/row descriptors.
            we_tiles = []
            for e in range(E):
                halves = []
                for j in range(JT):
                    wej = wpool.tile(
                        [128, KT * 512], bf16, tag="we", name=f"we{e}j{j}"
                    )
                    nc.sync.dma_start(wej[:], we_d[e, j])
                    halves.append(wej)
                we_tiles.append(halves)

            acc = apool.tile([128, TT * O], f32)

            g_f32 = gpool.tile([128, TT * E], f32)
            g_bf = gpool.tile([128, TT * E], bf16)
            gT = gpool.tile([E, NLOC], bf16)
            negm = gpool.tile([128, TT], f32)
            ssum = gpool.tile([128, TT], f32)
            rec = gpool.tile([128, TT], f32)

            def xt_tile(k, t):
                return xts[t][:, k * 128 : (k + 1) * 128]

            # ---- Phase A: gating logits + softmax --------------------------
            with (
                tc.tile_pool(name="psA", bufs=1, space="PSUM") as psA,
                tc.tile_pool(name="psC", bufs=1, space="PSUM") as psC,
                tc.tile_pool(name="psB", bufs=6, space="PSUM") as psB,
            ):
                for t in range(TT):
                    zg = psA.tile([128, E], f32, tag="zg")
                    for k in range(KT):
                        nc.tensor.matmul(
                            zg[:],
                            xt_tile(k, t),
                            wg_sb[:, k * E : (k + 1) * E],
                            start=(k == 0),
                            stop=False,
                        )
                    # + bg (rank-1: ones[1,128].T @ bg[1,E])
                    nc.tensor.matmul(zg[:], ones[:], bg_sb[:], start=False, stop=True)

                    nm = negm[:, t : t + 1]
                    nc.vector.tensor_reduce(
                        nm, zg[:], axis=mybir.AxisListType.X, op=Alu.max, negate=True
                    )
                    gs = g_f32[:, t * E : (t + 1) * E]
                    nc.scalar.activation(
                        gs,
                        zg[:],
                        mybir.ActivationFunctionType.Exp,
                        bias=nm,
                        scale=1.0,
                        accum_out=ssum[:, t : t + 1],
                    )
                    nc.vector.reciprocal(rec[:, t : t + 1], ssum[:, t : t + 1])
                    nc.vector.tensor_scalar_mul(gs, gs, rec[:, t : t + 1])
                    nc.vector.tensor_copy(g_bf[:, t * E : (t + 1) * E], gs)

                # ---- Phase A2: transpose gates for the bias matmul ---------
                for t in range(TT):
                    trp = psC.tile([E, 128], bf16, tag="tr")
                    nc.tensor.transpose(
                        trp[:], g_bf[:, t * E : (t + 1) * E], ident[:]
                    )
                    nc.vector.tensor_copy(gT[:, t * 128 : (t + 1) * 128], trp[:])

                # ---- Phase B: expert GEMMs + gated accumulate --------------
                # e == 0 writes acc (no init needed); e >= 1 run the fused
                # DVE accumulate acc = psum_e * g_e + acc; the bias term
                # g @ be is folded in during the last expert's pass.
                # e = 0: j-outer so the j=0 half of We[0] is consumed as
                # soon as its DMA lands.
                for j in range(JT):
                    we_sb = we_tiles[0][j]
                    for t in range(TT):
                        ps0 = psB.tile([128, 512], f32, tag="mm", name="mm0")
                        for k in range(KT):
                            nc.tensor.matmul(
                                ps0[:],
                                xt_tile(k, t),
                                we_sb[:, k * 512 : (k + 1) * 512],
                                start=(k == 0),
                                stop=(k == KT - 1),
                            )
                        nc.vector.tensor_scalar_mul(
                            acc[:, t * O + j * 512 : t * O + (j + 1) * 512],
                            ps0[:],
                            g_f32[:, t * E : t * E + 1],
                        )

            with (
                tc.tile_pool(name="psD", bufs=2, space="PSUM") as psD,
                tc.tile_pool(name="psB2", bufs=6, space="PSUM") as psB2,
            ):
                for e in range(1, E):
                    last = e == E - 1
                    for t in range(TT):
                        bps = []
                        if last:
                            # Bias term g @ be, folded into the final pass so
                            # its PSUM tiles are short-lived.
                            for j in range(JT):
                                bp = psD.tile([128, 512], f32, tag="bp", name=f"bp{j}")
                                nc.tensor.matmul(
                                    bp[:],
                                    gT[:, t * 128 : (t + 1) * 128],
                                    be_sb[:, j * 512 : (j + 1) * 512],
                                    start=True,
                                    stop=True,
                                )
                                bps.append(bp)
                        ps = [
                            psB2.tile([128, 512], f32, tag="mm", name=f"mm{j}")
                            for j in range(JT)
                        ]
                        for k in range(KT):
                            lhs = xt_tile(k, t)
                            for j in range(JT):
                                nc.tensor.matmul(
                                    ps[j][:],
                                    lhs,
                                    we_tiles[e][j][:, k * 512 : (k + 1) * 512],
                                    start=(k == 0),
                                    stop=(k == KT - 1),
                                )
                        gcol = g_f32[:, t * E + e : t * E + e + 1]
                        for j in range(JT):
                            a_sl = acc[:, t * O + j * 512 : t * O + (j + 1) * 512]
                            if last:
                                # Fold the bias in BEFORE the final expert's
                                # accumulate so the post-last-matmul critical
                                # path is one DVE op + the store.
                                nc.vector.scalar_tensor_tensor(
                                    a_sl, bps[j][:], 1.0, a_sl,
                                    op0=Alu.mult, op1=Alu.add,
                                )
                            nc.vector.scalar_tensor_tensor(
                                a_sl, ps[j][:], gcol, a_sl,
                                op0=Alu.mult, op1=Alu.add,
                            )
                            if last:
                                nc.sync.dma_start(
                                    out_d[
                                        t * 128 : (t + 1) * 128,
                                        j * 512 : (j + 1) * 512,
                                    ],
                                    a_sl,
                                )

    nc.compile()
    return nc


def _get_nc():
    if "nc" not in _CACHE:
        _CACHE["nc"] = _build()
    return _CACHE["nc"]


def _pack_inputs(x, We, be, Wg, bg):
    """Host-side packing: shard + pre-transpose + bf16 cast.

    Every packed layout is chosen so the device DMA descriptors are
    contiguous >=2KB rows.
    """
    x = np.asarray(x, dtype=np.float32)
    We = np.asarray(We, dtype=np.float32)
    be = np.asarray(be, dtype=np.float32)
    Wg = np.asarray(Wg, dtype=np.float32)
    bg = np.asarray(bg, dtype=np.float32)

    # we_p[e, j, p, k*512+c] = We[e][k*128+p, j*512+c]
    we_p = np.ascontiguousarray(
        We.reshape(E, KT, 128, JT, 512).transpose(0, 3, 2, 1, 4).reshape(
            E, JT, 128, KT * 512
        )
    ).astype(BF16)
    be_p = be.astype(BF16)
    wg_p = np.ascontiguousarray(
        Wg.reshape(KT, 128, E).transpose(1, 0, 2).reshape(128, KT * E)
    ).astype(BF16)
    bg_p = bg.reshape(1, E).astype(BF16)

    in_maps = []
    for i in range(NCORES):
        xs = x[i * NLOC : (i + 1) * NLOC]          # [NLOC, D]
        # xt[p, t*KT*128 + k*128 + n] = xs[t*128+n, k*128+p]
        xt = np.ascontiguousarray(
            xs.reshape(TT, 128, KT, 128).transpose(3, 0, 2, 1).reshape(
                128, TT * KT * 128
            )
        ).astype(BF16)
        in_maps.append(
            {"xt": xt, "Wep": we_p, "bep": be_p, "Wgp": wg_p, "bgp": bg_p}
        )
    return in_maps


def _run(inputs, trace=False):
    """Returns (y_full, BassKernelResults)."""
    from concourse.bass_utils import run_bass_kernel_spmd

    nc = _get_nc()
    in_maps = _pack_inputs(**inputs)
    res = run_bass_kernel_spmd(
        nc, in_maps, core_ids=list(range(NCORES)), trace=trace
    )
    y = np.concatenate(
        [res.results[i]["out"] for i in range(NCORES)], axis=0
    ).astype(np.float32)
    return y, res


def kernel(**inputs):
    y, _ = _run(inputs, trace=False)
    return y


# revision 4
# speedup vs baseline: 1.0278x; 1.0238x over previous
"""Trainium2 Bass kernel for BasicMoE — v3.

Reference computation (N=8192 tokens, D=1024 in, O=1024 out, E=8 experts):
    gates = softmax(x @ Wg + bg)                        # [N, E]
    out   = sum_e gates[:, e] * (x @ We[e] + be[e])     # [N, O]

Data-parallel over tokens: each core gets 1024 tokens + replicated weights.

v3 layout/schedule (from the v1/v2 traces):
  - The sync HWDGE ring moves first bytes ~0.7us after the ~8us engine
    preamble; the scalar ring lags ~3.5us more.  So everything start-
    latency-critical goes on the sync ring, in exact consumption order:
      [bg|Wg|xt chunk0] [We0j0 k-pairs x4] [xt chunks 1-7] [We0j1]
      [We1j0] ... [We7j1]  (+ output stores at the end)
  - bias path off the PE: be host-broadcast to [128, E*O]; per-expert bias
    lands as a second DVE scalar_tensor_tensor per (e,t,j); the 8 per-
    expert bias chunks stream on the (slow-starting, otherwise idle)
    scalar ring.
  - softmax without max-subtraction (logits are O(1); exp safe in f32).
  - gating(t) interleaved with expert-0 j=0 (t) in emission order so the
    PE fills the We0j0 DMA-pacing gaps with gating work.
"""

import numpy as np
import ml_dtypes

N_TOKENS = 8192
D = 1024   # in dim
O = 1024   # out dim
E = 8      # experts
NCORES = 8
NLOC = N_TOKENS // NCORES   # 1024 tokens per core
KT = D // 128               # 8 k-chunks
TT = NLOC // 128            # 8 token chunks
JT = O // 512               # 2 out chunks
HDR = 8 + KT * E            # bg(8) + Wg(64) columns prepended to chunk 0

BF16 = ml_dtypes.bfloat16

_CACHE = {}


def _build():
    """Build + compile the per-core Bass graph (same graph on all 8 cores)."""
    import concourse.bass as bass
    import concourse.mybir as mybir
    import concourse.tile as tile
    from concourse import bacc

    dt = mybir.dt
    f32 = dt.float32
    bf16 = dt.bfloat16
    Alu = mybir.AluOpType

    nc = bacc.Bacc(
        "TRN2",
        target_bir_lowering=False,
        debug=False,
        enable_asserts=False,
        num_devices=NCORES,
    )

    # xt: [bg(8) | Wg(64) | chunk0(1024) | chunk1..7(1024 each)], 2KB+ rows.
    xt_d = nc.dram_tensor(
        "xt", [128, HDR + TT * KT * 128], bf16, kind="ExternalInput"
    ).ap()
    # We: [e, j, p, k*512+c] — each (e, j) block is [128, KT*512], 8KB rows.
    we_d = nc.dram_tensor("Wep", [E, JT, 128, KT * 512], bf16, kind="ExternalInput").ap()
    # be broadcast across partitions on host: [128, e*O+o].
    bebc_d = nc.dram_tensor("bebc", [128, E * O], bf16, kind="ExternalInput").ap()
    out_d = nc.dram_tensor("out", [NLOC, O], f32, kind="ExternalOutput").ap()

    with tile.TileContext(nc) as tc:
        with (
            tc.tile_pool(name="const", bufs=1) as cpool,
            tc.tile_pool(name="xp", bufs=1) as xpool,
            tc.tile_pool(name="wp", bufs=6) as wpool,
            tc.tile_pool(name="ap", bufs=1) as apool,
            tc.tile_pool(name="gp", bufs=1) as gpool,
        ):
            ones = cpool.tile([1, 128], bf16)
            nc.gpsimd.memset(ones[:], 1.0)

            # --- sync-ring queue, in consumption order ------------------
            xc0 = xpool.tile([128, HDR + KT * 128], bf16, name="xc0")
            nc.sync.dma_start(xc0[:], xt_d[:, 0 : HDR + KT * 128])

            we_tiles = [
                [
                    wpool.tile([128, KT * 512], bf16, tag="we", name=f"we{e}j{j}")
                    for j in range(JT)
                ]
                for e in range(E)
            ]
            # We[0] j=0 as 4 k-pair chunks (2KB-row descriptors).
            for kk in range(0, KT, 2):
                nc.sync.dma_start(
                    we_tiles[0][0][:, kk * 512 : (kk + 2) * 512],
                    we_d[0, 0][:, kk * 512 : (kk + 2) * 512],
                )
            xts = [xc0]
            for t in range(1, TT):
                xc = xpool.tile([128, KT * 128], bf16, tag="xt", bufs=TT - 1,
                                name=f"xt{t}")
                nc.sync.dma_start(
                    xc[:],
                    xt_d[:, HDR + t * KT * 128 : HDR + (t + 1) * KT * 128],
                )
                xts.append(xc)
            nc.sync.dma_start(we_tiles[0][1][:], we_d[0, 1])
            for e in range(1, E):
                for j in range(JT):
                    nc.sync.dma_start(we_tiles[e][j][:], we_d[e, j])

            # --- scalar ring: per-expert broadcast-bias chunks ----------
            bebc = cpool.tile([128, E * O], bf16)
            for e in range(E):
                nc.scalar.dma_start(
                    bebc[:, e * O : (e + 1) * O],
                    bebc_d[:, e * O : (e + 1) * O],
                )

            acc = apool.tile([128, TT * O], f32)
            g_f32 = gpool.tile([128, TT * E], f32)
            ssum = gpool.tile([128, TT], f32)
            rec = gpool.tile([128, TT], f32)

            bg_ap = xc0[0:1, 0:E]

            def wg_sl(k):
                return xc0[:, 8 + k * E : 8 + (k + 1) * E]

            def xt_tile(k, t):
                if t == 0:
                    return xc0[:, HDR + k * 128 : HDR + (k + 1) * 128]
                return xts[t][:, k * 128 : (k + 1) * 128]

            def gcol(t, e):
                return g_f32[:, t * E + e : t * E + e + 1]

            def acc_sl(t, j):
                return acc[:, t * O + j * 512 : t * O + (j + 1) * 512]

            def be_sl(e, j):
                return bebc[:, e * O + j * 512 : e * O + (j + 1) * 512]

            with (
                tc.tile_pool(name="psA", bufs=1, space="PSUM") as psA,
                tc.tile_pool(name="psB", bufs=7, space="PSUM") as psB,
            ):
                def gating(t):
                    zg = psA.tile([128, E], f32, tag="zg", name="zg")
                    for k in range(KT):
                        nc.tensor.matmul(
                            zg[:], xt_tile(k, t), wg_sl(k),
                            start=(k == 0), stop=False,
                        )
                    # + bg (rank-1: ones[1,128].T @ bg[1,E])
                    nc.tensor.matmul(zg[:], ones[:], bg_ap, start=False, stop=True)
                    gs = g_f32[:, t * E : (t + 1) * E]
                    # No max-subtraction: logits are O(1) here, exp is safe.
                    nc.scalar.activation(
                        gs, zg[:], mybir.ActivationFunctionType.Exp,
                        accum_out=ssum[:, t : t + 1],
                    )
                    nc.vector.reciprocal(rec[:, t : t + 1], ssum[:, t : t + 1])
                    nc.vector.tensor_scalar_mul(gs, gs, rec[:, t : t + 1])

                def expert_tj(e, t, j):
                    """k-loop matmuls + gated accumulate for one (e, t, j)."""
                    last = e == E - 1
                    ps = psB.tile([128, 512], f32, tag="mm", name="mm")
                    if e > 0:
                        # Bias add first: it only depends on the previous
                        # expert's acc, so the DVE does it while the PE is
                        # still streaming this expert's matmuls.
                        nc.vector.scalar_tensor_tensor(
                            acc_sl(t, j), be_sl(e, j), gcol(t, e), acc_sl(t, j),
                            op0=Alu.mult, op1=Alu.add,
                        )
                    for k in range(KT):
                        nc.tensor.matmul(
                            ps[:],
                            xt_tile(k, t),
                            we_tiles[e][j][:, k * 512 : (k + 1) * 512],
                            start=(k == 0),
                            stop=(k == KT - 1),
                        )
                    if e == 0:
                        nc.vector.tensor_scalar_mul(acc_sl(t, j), ps[:], gcol(t, 0))
                        nc.vector.scalar_tensor_tensor(
                            acc_sl(t, j), be_sl(0, j), gcol(t, 0), acc_sl(t, j),
                            op0=Alu.mult, op1=Alu.add,
                        )
                    else:
                        nc.vector.scalar_tensor_tensor(
                            acc_sl(t, j), ps[:], gcol(t, e), acc_sl(t, j),
                            op0=Alu.mult, op1=Alu.add,
                        )
                    if last:
                        nc.sync.dma_start(
                            out_d[t * 128 : (t + 1) * 128, j * 512 : (j + 1) * 512],
                            acc_sl(t, j),
                        )

                # Interleave gating with expert-0 j=0: the gating matmuls
                # fill the We0j0/xt DMA-pacing gaps.
                for t in range(TT):
                    gating(t)
                    expert_tj(0, t, 0)
                for t in range(TT):
                    expert_tj(0, t, 1)
                for e in range(1, E):
                    for t in range(TT):
                        for j in range(JT):
                            expert_tj(e, t, j)

    nc.compile()
    return nc


def _get_nc():
    if "nc" not in _CACHE:
        _CACHE["nc"] = _build()
    return _CACHE["nc"]


def _pack_inputs(x, We, be, Wg, bg):
    """Host-side packing: shard + pre-transpose + bf16 cast.

    Every packed layout is chosen so the device DMA descriptors are
    contiguous >=2KB rows.
    """
    x = np.asarray(x, dtype=np.float32)
    We = np.asarray(We, dtype=np.float32)
    be = np.asarray(be, dtype=np.float32)
    Wg = np.asarray(Wg, dtype=np.float32)
    bg = np.asarray(bg, dtype=np.float32)

    # we_p[e, j, p, k*512+c] = We[e][k*128+p, j*512+c]
    we_p = np.ascontiguousarray(
        We.reshape(E, KT, 128, JT, 512).transpose(0, 3, 2, 1, 4).reshape(
            E, JT, 128, KT * 512
        )
    ).astype(BF16)
    bebc = np.ascontiguousarray(
        np.tile(be.reshape(1, E * O), (128, 1))
    ).astype(BF16)
    # header: bg broadcast (8 cols) | wg[p, k*E+e] (64 cols)
    bg_bc = np.tile(bg.reshape(1, E), (128, 1)).astype(np.float32)
    wg_p = Wg.reshape(KT, 128, E).transpose(1, 0, 2).reshape(128, KT * E)

    in_maps = []
    for i in range(NCORES):
        xs = x[i * NLOC : (i + 1) * NLOC]          # [NLOC, D]
        # xt[p, t*KT*128 + k*128 + n] = xs[t*128+n, k*128+p]
        xt = xs.reshape(TT, 128, KT, 128).transpose(3, 0, 2, 1).reshape(
            128, TT * KT * 128
        )
        xt_full = np.ascontiguousarray(
            np.concatenate([bg_bc, wg_p, xt], axis=1)
        ).astype(BF16)
        in_maps.append({"xt": xt_full, "Wep": we_p, "bebc": bebc})
    return in_maps


def _run(inputs, trace=False):
    """Returns (y_full, BassKernelResults)."""
    from concourse.bass_utils import run_bass_kernel_spmd

    nc = _get_nc()
    in_maps = _pack_inputs(**inputs)
    res = run_bass_kernel_spmd(
        nc, in_maps, core_ids=list(range(NCORES)), trace=trace
    )
    y = np.concatenate(
        [res.results[i]["out"] for i in range(NCORES)], axis=0
    ).astype(np.float32)
    return y, res


def kernel(**inputs):
    y, _ = _run(inputs, trace=False)
    return y


# revision 10
# speedup vs baseline: 1.0450x; 1.0167x over previous
"""Trainium2 Bass kernel for BasicMoE — v5.

Reference computation (N=8192 tokens, D=1024 in, O=1024 out, E=8 experts):
    gates = softmax(x @ Wg + bg)                        # [N, E]
    out   = sum_e gates[:, e] * (x @ We[e] + be[e])     # [N, O]

Data-parallel over tokens: each core gets 1024 tokens + replicated weights;
no collectives.  The per-core kernel is a single near-zero-bubble PE stream
of 512 bf16 matmuls (K=128, N=1024) at the warm issue floor (~429ns each).

Key design points (each from a measured trace):
  - All start-latency-critical DMAs ride the sync HWDGE ring (first bytes
    ~0.7us after the ~8us engine preamble; the scalar ring lags ~3.5us) in
    exact consumption order: [bg|Wg|xt chunk0], We[0] as 4 k-pair chunks,
    xt chunks 1-3, broadcast-bias, We[1..7] (1 contiguous 2MB DMA each,
    16KB-row descriptors), output stores.  xt chunks 4-7 take the scalar
    ring.  Loading the 2MB bias up-front would steal ~40% of the early HBM
    budget from the critical path (measured), hence its late slot.
  - Matmuls accumulate f32 into a two-bank [128,1024] psum tile; the DVE
    drains both halves with ONE scalar_tensor_tensor per (e,t), and acc /
    bias / output are bf16 (the all-bf16 bias add gets the 2x-packed DVE
    mode; the output upcasts on host).  Host-simulated rel_err of a fully
    bf16 accumulate chain is 6.6e-3 vs the 2e-2 gate; this (f32 psum)
    variant is strictly more accurate.
  - gating(t) is interleaved with expert-0(t) in emission order so the PE
    fills the early DMA-pacing gaps with gating work; softmax skips
    max-subtraction (logits are O(1)) and exp's accum_out gives the row
    sums for free.
"""

import numpy as np
import ml_dtypes

N_TOKENS = 8192
D = 1024   # in dim
O = 1024   # out dim
E = 8      # experts
NCORES = 8
NLOC = N_TOKENS // NCORES   # 1024 tokens per core
KT = D // 128               # 8 k-chunks
TT = NLOC // 128            # 8 token chunks
HDR = 8 + KT * E            # bg(8) + Wg(64) columns prepended to chunk 0

BF16 = ml_dtypes.bfloat16

_CACHE = {}


def _build():
    """Build + compile the per-core Bass graph (same graph on all 8 cores)."""
    import concourse.bass as bass
    import concourse.mybir as mybir
    import concourse.tile as tile
    from concourse import bacc

    dt = mybir.dt
    f32 = dt.float32
    bf16 = dt.bfloat16
    Alu = mybir.AluOpType

    nc = bacc.Bacc(
        "TRN2",
        target_bir_lowering=False,
        debug=False,
        enable_asserts=False,
        num_devices=NCORES,
    )

    # xt: [bg(8) | Wg(64) | chunk0(1024) | chunk1..7(1024 each)], 2KB+ rows.
    xt_d = nc.dram_tensor(
        "xt", [128, HDR + TT * KT * 128], bf16, kind="ExternalInput"
    ).ap()
    # We: [e, p, k*1024+o] — each expert is [128, KT*O], 16KB rows.
    we_d = nc.dram_tensor("Wep", [E, 128, KT * O], bf16, kind="ExternalInput").ap()
    # be broadcast across partitions on host: [128, e*O+o].
    bebc_d = nc.dram_tensor("bebc", [128, E * O], bf16, kind="ExternalInput").ap()
    out_d = nc.dram_tensor("out", [NLOC, O], bf16, kind="ExternalOutput").ap()

    with tile.TileContext(nc) as tc:
        with (
            tc.tile_pool(name="const", bufs=1) as cpool,
            tc.tile_pool(name="xp", bufs=1) as xpool,
            tc.tile_pool(name="wp", bufs=4) as wpool,
            tc.tile_pool(name="ap", bufs=1) as apool,
            tc.tile_pool(name="gp", bufs=1) as gpool,
        ):
            ones = cpool.tile([1, 128], bf16)
            nc.gpsimd.memset(ones[:], 1.0)

            # --- sync-ring queue, in consumption order ------------------
            xc0 = xpool.tile([128, HDR + KT * 128], bf16, name="xc0")
            nc.sync.dma_start(xc0[:], xt_d[:, 0 : HDR + KT * 128])

            we_tiles = [
                wpool.tile([128, KT * O], bf16, tag="we", name=f"we{e}")
                for e in range(E)
            ]
            # We[0] as 4 k-pair chunks (4KB-row descriptors) so the first
            # expert matmul can start ~2us after xt chunk 0 lands.
            for kk in range(0, KT, 2):
                nc.sync.dma_start(
                    we_tiles[0][:, kk * O : (kk + 2) * O],
                    we_d[0][:, kk * O : (kk + 2) * O],
                )
            xts = [xc0]
            for t in range(1, TT):
                xc = xpool.tile([128, KT * 128], bf16, tag="xt", bufs=TT - 1,
                                name=f"xt{t}")
                xts.append(xc)
            # xt chunks 1-3 on the sync ring (needed early), 4-7 on the
            # scalar ring (starts ~3.5us later; needed later anyway).
            for t in range(1, TT):
                eng = nc.sync if t < 4 else nc.scalar
                eng.dma_start(
                    xts[t][:],
                    xt_d[:, HDR + t * KT * 128 : HDR + (t + 1) * KT * 128],
                )
            # Broadcast bias after the xt chunks: first needed by the DVE
            # at ~15us, lands ~14us; up-front it would halve the early HBM
            # bandwidth of the latency-critical transfers above.
            bebc = cpool.tile([128, E * O], bf16)
            for be_ in range(E):
                nc.sync.dma_start(
                    bebc[:, be_ * O : (be_ + 1) * O],
                    bebc_d[:, be_ * O : (be_ + 1) * O],
                )
            for e in range(1, E):
                nc.sync.dma_start(we_tiles[e][:], we_d[e])

            acc = apool.tile([128, TT * O], bf16)
            g_f32 = gpool.tile([128, TT * E], f32)
            ssum = gpool.tile([128, TT], f32)
            rec = gpool.tile([128, TT], f32)

            bg_ap = xc0[0:1, 0:E]

            def wg_sl(k):
                return xc0[:, 8 + k * E : 8 + (k + 1) * E]

            def xt_tile(k, t):
                if t == 0:
                    return xc0[:, HDR + k * 128 : HDR + (k + 1) * 128]
                return xts[t][:, k * 128 : (k + 1) * 128]

            def gcol(t, e):
                return g_f32[:, t * E + e : t * E + e + 1]

            def acc_sl(t):
                return acc[:, t * O : (t + 1) * O]

            def be_sl(e):
                return bebc[:, e * O : (e + 1) * O]

            with (
                tc.tile_pool(name="psA", bufs=1, space="PSUM") as psA,
                tc.tile_pool(name="psB", bufs=3, space="PSUM") as psB,
            ):
                def gating(t):
                    zg = psA.tile([128, E], f32, tag="zg", name="zg")
                    for k in range(KT):
                        nc.tensor.matmul(
                            zg[:], xt_tile(k, t), wg_sl(k),
                            start=(k == 0), stop=False,
                        )
                    # + bg (rank-1: ones[1,128].T @ bg[1,E])
                    nc.tensor.matmul(zg[:], ones[:], bg_ap, start=False, stop=True)
                    gs = g_f32[:, t * E : (t + 1) * E]
                    # No max-subtraction: logits are O(1) here, exp is safe.
                    nc.scalar.activation(
                        gs, zg[:], mybir.ActivationFunctionType.Exp,
                        accum_out=ssum[:, t : t + 1],
                    )
                    nc.vector.reciprocal(rec[:, t : t + 1], ssum[:, t : t + 1])
                    nc.vector.tensor_scalar_mul(gs, gs, rec[:, t : t + 1])

                def expert_t(e, t):
                    """k-loop matmuls + gated accumulate + bias for (e, t).

                    The psum tile spans two banks; each matmul targets one
                    512-col bank slice (matmul out must be f32 and fit one
                    bank), but the DVE drains both halves in one op.
                    """
                    last = e == E - 1
                    ps = psB.tile([128, O], f32, tag="mm", name="mm")
                    if e > 0:
                        # Bias add first: it only depends on the previous
                        # expert's acc, so the DVE runs it while the PE is
                        # still streaming this expert's matmuls.
                        nc.vector.scalar_tensor_tensor(
                            acc_sl(t), be_sl(e), gcol(t, e), acc_sl(t),
                            op0=Alu.mult, op1=Alu.add,
                        )
                    for k in range(KT):
                        lhs = xt_tile(k, t)
                        for j in range(2):
                            nc.tensor.matmul(
                                ps[:, j * 512 : (j + 1) * 512],
                                lhs,
                                we_tiles[e][:, k * O + j * 512 : k * O + (j + 1) * 512],
                                start=(k == 0),
                                stop=(k == KT - 1),
                            )
                    if e == 0:
                        nc.vector.tensor_scalar_mul(acc_sl(t), ps[:], gcol(t, 0))
                        nc.vector.scalar_tensor_tensor(
                            acc_sl(t), be_sl(0), gcol(t, 0), acc_sl(t),
                            op0=Alu.mult, op1=Alu.add,
                        )
                    else:
                        nc.vector.scalar_tensor_tensor(
                            acc_sl(t), ps[:], gcol(t, e), acc_sl(t),
                            op0=Alu.mult, op1=Alu.add,
                        )
                    if last:
                        nc.sync.dma_start(
                            out_d[t * 128 : (t + 1) * 128, :], acc_sl(t)
                        )

                # Interleave gating with expert 0: the gating matmuls fill
                # the We[0]/xt DMA-pacing gaps at the start.
                for t in range(TT):
                    gating(t)
                    expert_t(0, t)
                for e in range(1, E):
                    for t in range(TT):
                        expert_t(e, t)

    nc.compile()
    return nc


def _get_nc():
    if "nc" not in _CACHE:
        _CACHE["nc"] = _build()
    return _CACHE["nc"]


def _pack_inputs(x, We, be, Wg, bg):
    """Host-side packing: shard + pre-transpose + bf16 cast.

    Every packed layout is chosen so the device DMA descriptors are
    contiguous >=2KB rows.
    """
    x = np.asarray(x, dtype=np.float32)
    We = np.asarray(We, dtype=np.float32)
    be = np.asarray(be, dtype=np.float32)
    Wg = np.asarray(Wg, dtype=np.float32)
    bg = np.asarray(bg, dtype=np.float32)

    # we_p[e, p, k*O+o] = We[e][k*128+p, o]
    we_p = np.ascontiguousarray(
        We.reshape(E, KT, 128, O).transpose(0, 2, 1, 3).reshape(E, 128, KT * O)
    ).astype(BF16)
    bebc = np.ascontiguousarray(
        np.tile(be.reshape(1, E * O), (128, 1))
    ).astype(BF16)
    # header: bg broadcast (8 cols) | wg[p, k*E+e] (64 cols)
    bg_bc = np.tile(bg.reshape(1, E), (128, 1)).astype(np.float32)
    wg_p = Wg.reshape(KT, 128, E).transpose(1, 0, 2).reshape(128, KT * E)

    in_maps = []
    for i in range(NCORES):
        xs = x[i * NLOC : (i + 1) * NLOC]          # [NLOC, D]
        # xt[p, t*KT*128 + k*128 + n] = xs[t*128+n, k*128+p]
        xt = xs.reshape(TT, 128, KT, 128).transpose(3, 0, 2, 1).reshape(
            128, TT * KT * 128
        )
        xt_full = np.ascontiguousarray(
            np.concatenate([bg_bc, wg_p, xt], axis=1)
        ).astype(BF16)
        in_maps.append({"xt": xt_full, "Wep": we_p, "bebc": bebc})
    return in_maps


def _run(inputs, trace=False):
    """Returns (y_full, BassKernelResults)."""
    from concourse.bass_utils import run_bass_kernel_spmd

    nc = _get_nc()
    in_maps = _pack_inputs(**inputs)
    res = run_bass_kernel_spmd(
        nc, in_maps, core_ids=list(range(NCORES)), trace=trace
    )
    y = np.concatenate(
        [res.results[i]["out"] for i in range(NCORES)], axis=0
    ).astype(np.float32)
    return y, res


def kernel(**inputs):
    y, _ = _run(inputs, trace=False)
    return y
